# revision 18
# baseline (speedup 1.0000x reference)
"""Causal self-attention Trainium2 kernel (8 NeuronCores, SPMD).

Sharding: data-parallel over batch (B=2) x tensor-parallel over heads
(16 heads -> 4 per core).  core c: batch c//4, heads 4*(c%4) .. +4.
Each core computes qkv projection for its heads, causal attention, and a
partial out-projection; the host sums the 4 head-group partials per batch.

Layout notes:
  - Host passes x[b] pre-transposed (C, T) so the contraction dim C is
    partition-major for the qkv matmuls (PE contracts over partitions).
  - q,k are produced transposed (head_dim, T); v in natural (T, head_dim)
    with a ones column appended so the AV matmul also emits softmax row
    sums (row 64 of the PSUM accumulator).
  - scores are computed transposed (k, q) so the exp'd weights feed the
    AV matmul directly as the moving operand.
  - All matmul operands are bfloat16 (full-rate PE with fast weight
    load; fp32r pays a slow LDWEIGHTS and a 4x penalty at moving<256).
    PSUM accumulation stays fp32; rel err ~5e-3 vs the 2e-2 gate.
  - Static inputs live in consolidated tiles loaded with a few merged
    3D-AP DMAs (a DMA-issue instruction costs ~0.6us of engine time, so
    40 small loads would serialize the startup).
  - The PE instruction queue is in-order, so PE filler units are only
    scheduled into attention chunks whose input DMAs have landed.
  - Causal masking multiplies the exp'd diagonal blocks by a 0/1
    triangle on the (otherwise idle) GpSimd engine.
  - The partial y is stored bf16 (half the store traffic); the host
    upcasts and reduces in fp32.
"""

import numpy as np
import ml_dtypes

import concourse.bass as bass
import concourse.mybir as mybir
import concourse.tile as tile
from concourse import bacc
from concourse import bass_utils

# Problem shape (hardcoded per spec)
B, T, C = 2, 2048, 1024
NH, HD = 16, 64
NCORES = 8
HPC = 4                      # heads per core
P = 128                      # partitions
CB = C // P                  # 8 contraction blocks
QCW = 512                    # query chunk width
NQC = T // QCW               # 4 query chunks
NKB = T // P                 # 16 key blocks
SCALE = 1.0 / 8.0            # 1/sqrt(HD)
WQKC = 2 * HPC * HD          # wqk columns (512)
WVC = HPC * HD               # wv columns (256)
VW = HPC * (HD + 1)          # v tile width incl. ones cols (260)

F32 = mybir.dt.float32
BF16 = mybir.dt.bfloat16
EXP = mybir.ActivationFunctionType.Exp
BF16_NP = ml_dtypes.bfloat16


def build_program():
    nc = bacc.Bacc("TRN2", target_bir_lowering=False, debug=False,
                   num_devices=NCORES)

    xT = nc.dram_tensor("xT", [C, T], BF16, kind="ExternalInput").ap()
    wqk = nc.dram_tensor("wqk", [C, WQKC], BF16, kind="ExternalInput").ap()
    wv = nc.dram_tensor("wv", [C, WVC], BF16, kind="ExternalInput").ap()
    wo = nc.dram_tensor("wo", [HPC * HD, C], BF16, kind="ExternalInput").ap()
    mask = nc.dram_tensor("mask", [P, P], BF16, kind="ExternalInput").ap()
    ones = nc.dram_tensor("ones", [P, HD], BF16, kind="ExternalInput").ap()
    y = nc.dram_tensor("y", [T, C], BF16, kind="ExternalOutput").ap()

    # slab-major dram views for merged loads: [p, cb, cols]
    xT_v = xT.rearrange("(cb p) t -> p cb t", p=P)
    wqk_v = wqk.rearrange("(cb p) c -> p cb c", p=P)
    wv_v = wv.rearrange("(cb p) c -> p cb c", p=P)
    wo_v = wo.rearrange("(hp p) c -> p hp c", p=P)

    with tile.TileContext(nc) as tc:
        with tc.tile_pool(name="sb", bufs=1) as sb, \
             tc.tile_pool(name="work", bufs=1) as work, \
             tc.tile_pool(name="dr", bufs=1, space="DRAM") as dr, \
             tc.tile_pool(name="ps", bufs=1, space="PSUM") as ps:

            # ---- consolidated static tiles
            xT_all = sb.tile([P, CB * T], BF16, tag="xT", bufs=1,
                             name="xT_all")
            xT_av = xT_all.rearrange("p (cb t) -> p cb t", t=T)
            xT_sb = [xT_all[:, cb * T:(cb + 1) * T] for cb in range(CB)]
            wqk_all = sb.tile([P, CB * WQKC], BF16, tag="wqk", bufs=1,
                              name="wqk_all")
            wqk_av = wqk_all.rearrange("p (cb c) -> p cb c", c=WQKC)
            wqk_sb = [wqk_all[:, cb * WQKC:(cb + 1) * WQKC] for cb in range(CB)]
            wv_all = sb.tile([P, CB * WVC], BF16, tag="wv", bufs=1,
                             name="wv_all")
            wv_av = wv_all.rearrange("p (cb c) -> p cb c", c=WVC)
            wv_sb = [wv_all[:, cb * WVC:(cb + 1) * WVC] for cb in range(CB)]
            wo_all = sb.tile([P, 2 * C], BF16, tag="wo", bufs=1,
                             name="wo_all")
            wo_av = wo_all.rearrange("p (hp c) -> p hp c", c=C)
            wo_sb = [wo_all[:, hp * C:(hp + 1) * C] for hp in range(2)]

            # ---- startup loads: the first attention chunk needs wqk and
            # xT cols 0:512; stream those in 0.25MB quarters round-robined
            # over all three DMA queues so arrival is progressive and no
            # single queue serializes the transfers.
            rr = [nc.sync, nc.scalar, nc.gpsimd]
            for q in range(4):
                cbs = slice(2 * q, 2 * q + 2)
                rr[(2 * q) % 3].dma_start(wqk_av[:, cbs, :],
                                          wqk_v[:, cbs, :])
                rr[(2 * q + 1) % 3].dma_start(xT_av[:, cbs, 0:QCW],
                                              xT_v[:, cbs, 0:QCW])
            # then, in priority order: v weights, x chunk 1, out weights,
            # constants, x chunk 2; x chunk 3 is prefetched after attention
            # chunk 0 is emitted
            nc.gpsimd.dma_start(wv_av, wv_v)
            nc.scalar.dma_start(xT_av[:, :, QCW:2 * QCW],
                                xT_v[:, :, QCW:2 * QCW])
            nc.sync.dma_start(wo_av, wo_v)
            ones_sb = sb.tile([P, HD], BF16, tag="ones", bufs=1)
            nc.sync.dma_start(ones_sb, ones)
            mask_sb = sb.tile([P, P], BF16, tag="mask", bufs=1)
            nc.sync.dma_start(mask_sb, mask)
            # x chunks 2+3 as one load: 2KB contiguous runs transfer more
            # efficiently, and both land long before their consumers
            nc.scalar.dma_start(xT_av[:, :, 2 * QCW:4 * QCW],
                                xT_v[:, :, 2 * QCW:4 * QCW])
            # fp32 ones row for the endgame K=1 broadcast matmuls
            ones32 = sb.tile([1, HD], F32, tag="ones32", bufs=1)
            nc.vector.memset(ones32, 1.0)
            # warm the exp table early (one tiny activation)
            exp_warm = sb.tile([1, HD], F32, tag="expwarm", bufs=1)
            nc.scalar.activation(exp_warm, ones32, EXP)

            # dependency-free micro-matmul into a scratch PSUM slot: placed
            # through the endgame so the HAM clock gate never sees a >3.4us
            # PE-idle window and the tail matmuls run at 2.4GHz.  PSUM is
            # full, so the scratch lazily borrows a ps2-pool slot (free by
            # the time the endgame runs).
            warm_state = {}

            def keep_warm():
                if 'ps' not in warm_state:
                    warm_state['ps'] = ps.tile([P, 2 * QCW], F32, tag="ps2",
                                               bufs=2, name="ps_warm")
                nc.tensor.matmul(warm_state['ps'][0:HD, 0:HD], ones32,
                                 ones32, start=True, stop=True)

            # ---- qkv projection ----
            # qk transposed, consolidated: col block jb*T+t;
            # jb 0,1 = q head pairs, 2,3 = k
            qk_all = sb.tile([P, 4 * T], BF16, tag="qk", bufs=1,
                             name="qk_all")
            qk_sb = [qk_all[:, jb * T:(jb + 1) * T] for jb in range(4)]
            # v natural per t-block, 4 heads x (64 v cols + ones col);
            # the ones cols never change -- set them once with a strided
            # memset instead of per-chunk copies
            v_all = sb.tile([P, NKB * VW], BF16, tag="v", bufs=1,
                            name="v_all")
            v_av = v_all.rearrange("p (tb h e) -> p tb h e", h=HPC, e=HD + 1)
            v_sb = [v_all[:, tb * VW:(tb + 1) * VW] for tb in range(NKB)]
            nc.gpsimd.memset(v_av[:, :, :, HD:HD + 1], 1.0)

            def qk_unit(tcg, jb):
                tsl = slice(tcg * QCW, (tcg + 1) * QCW)

                def emit():
                    ps_qk = ps.tile([P, QCW], F32, tag="ps", bufs=4,
                                    name="ps_qk")
                    for cb in range(CB):
                        nc.tensor.matmul(
                            ps_qk,
                            wqk_sb[cb][:, jb * P:(jb + 1) * P],
                            xT_sb[cb][:, tsl],
                            start=(cb == 0), stop=(cb == CB - 1))
                    nc.vector.tensor_copy(qk_sb[jb][:, tsl], ps_qk)
                return emit

            def v_unit(tcg, tbl):
                def emit():
                    tb = tcg * 4 + tbl
                    ps_v = ps.tile([P, HPC * HD], F32, tag="ps", bufs=4,
                                   name="ps_v")
                    for cb in range(CB):
                        nc.tensor.matmul(
                            ps_v,
                            xT_sb[cb][:, tb * P:(tb + 1) * P],
                            wv_sb[cb],
                            start=(cb == 0), stop=(cb == CB - 1))
                    nc.vector.tensor_copy(
                        v_av[:, tb, :, 0:HD],
                        ps_v.rearrange("p (h e) -> p h e", e=HD))
                return emit

            def qkv_units(tcg):
                return [qk_unit(tcg, jb) for jb in range(4)] + \
                       [v_unit(tcg, tbl) for tbl in range(4)]

            def outproj_units(qc, attn, final=False):
                def op_unit(tbl):
                    def emit():
                        tb = qc * 4 + tbl
                        out_sb = work.tile([P, C], BF16, tag="outsb", bufs=2,
                                           name="out_sb")
                        for cob in range(2):
                            ps_o = ps.tile([P, QCW], F32, tag="ps", bufs=4,
                                           name="ps_o")
                            for hp in range(2):
                                nc.tensor.matmul(
                                    ps_o,
                                    attn[hp][:, tbl * P:(tbl + 1) * P],
                                    wo_sb[hp][:, cob * QCW:(cob + 1) * QCW],
                                    start=(hp == 0), stop=(hp == 1))
                            osl = out_sb[:, cob * QCW:(cob + 1) * QCW]
                            if final:
                                # tail: ACT is idle (exps done) -- split the
                                # copies across both engines and store each
                                # half as soon as it lands
                                if cob == 0:
                                    nc.scalar.copy(osl, ps_o)
                                else:
                                    nc.vector.tensor_copy(osl, ps_o)
                                seng = nc.sync if tbl % 2 == 0 else nc.scalar
                                seng.dma_start(
                                    y[tb * P:(tb + 1) * P,
                                      cob * QCW:(cob + 1) * QCW], osl)
                            else:
                                nc.vector.tensor_copy(osl, ps_o)
                        if not final:
                            nc.sync.dma_start(y[tb * P:(tb + 1) * P, :],
                                              out_sb)
                    return emit
                return [op_unit(tbl) for tbl in range(4)]

            def emit_attention(qc, filler, last=False):
                """scores -> fused exp -> AV, with PE filler interleaved to
                keep TensorE dense (HAM warm) while ACT chews the exps."""
                attn = {}
                nkb = (qc + 1) * 4
                n_units = 2 * nkb
                fq = list(filler)
                credit = 0.0
                rate = len(fq) / n_units if n_units else 0.0
                for hp in range(2):
                    at = work.tile([P, QCW], BF16, tag="attn", bufs=6,
                                   name=f"attn_hp{hp}")
                    attn[hp] = at
                    ps_av = {}
                    for par in range(2):
                        ps_av[par] = ps.tile([P, QCW], F32, tag="ps", bufs=4,
                                             name="ps_av")
                    pend = []            # (kb, wexp2, jofs, w) awaiting AV
                    for kb in range(nkb):
                        kofs = kb - qc * 4
                        jofs = max(kofs, 0) * P
                        w = QCW - jofs
                        ps_s2 = ps.tile([P, 2 * QCW], F32, tag="ps2", bufs=2,
                                        name="ps_s2")
                        wexp2 = work.tile([P, 2 * QCW], BF16, tag="wexp2",
                                          bufs=5, name="wexp2")
                        diag = kofs >= 0
                        for par in range(2):
                            po = par * HD
                            nc.tensor.matmul(
                                ps_s2[:, par * QCW:par * QCW + w],
                                qk_sb[2 + hp][po:po + HD, kb * P:(kb + 1) * P],
                                qk_sb[hp][po:po + HD,
                                          qc * QCW + jofs:(qc + 1) * QCW],
                                start=True, stop=True)
                        sview = ps_s2.rearrange("p (g q) -> p g q", g=2)
                        wview = wexp2.rearrange("p (g q) -> p g q", g=2)
                        nc.scalar.activation(wview[:, :, 0:w],
                                             sview[:, :, 0:w], EXP, scale=SCALE)
                        if diag:
                            # zero the masked triangle on the idle GpSimd
                            # engine (frees the PE of 64 mask matmuls)
                            for par in range(2):
                                msl = wexp2[:, par * QCW:par * QCW + P]
                                nc.gpsimd.tensor_mul(msl, msl, mask_sb)
                        pend.append((kb, wexp2, jofs, w))
                        if len(pend) > 3:   # AV lags scores by 3 kb
                            _emit_av(hp, ps_av, pend.pop(0), nkb)
                        # drain PE filler to keep TensorE busy during exp
                        credit += rate
                        while credit >= 1.0 and fq:
                            fq.pop(0)()
                            credit -= 1.0
                    while pend:
                        _emit_av(hp, ps_av, pend.pop(0), nkb)
                        if fq:
                            fq.pop(0)()
                        elif last and hp == 1:
                            keep_warm()
                    # stage AV out of PSUM promptly so the banks recycle;
                    # heads land at their attn-aligned partition offsets
                    av_st = work.tile([P, QCW], BF16, tag="avst",
                                      bufs=4, name="av_st")
                    sg = work.tile([2, QCW], F32, tag="sumg", bufs=2,
                                   name="sums_g")
                    s1 = work.tile([1, QCW], F32, tag="sum1", bufs=2,
                                   name="sums_1")
                    rg = work.tile([2, QCW], F32, tag="recg", bufs=2,
                                   name="rec_g")
                    if last and hp == 1:
                        # endgame: ACT (done with exps) stages both av
                        # halves while DVE does sums + reciprocal; no DMA
                        # sums-gather, no bf16 casts; broadcast via K=1
                        # fp32 ones-row matmuls on the (idle) PE.  First
                        # 128 columns go alone so the out-projection tail
                        # starts immediately.
                        nc.scalar.copy(av_st[0:HD, :], ps_av[0][0:HD, :])
                        nc.scalar.copy(av_st[HD:P, :], ps_av[1][0:HD, :])
                        nc.vector.tensor_copy(sg[0:1, :],
                                              ps_av[0][HD:HD + 1, :])
                        nc.vector.tensor_copy(s1, ps_av[1][HD:HD + 1, :])
                        r1 = work.tile([1, QCW], F32, tag="rec1", bufs=1,
                                       name="rec_1")
                        keep_warm()
                        for csl in (slice(0, P), slice(P, QCW)):
                            nc.vector.reciprocal_approx_fast(
                                rg[0:1, csl], sg[0:1, csl])
                            nc.vector.reciprocal_approx_fast(
                                r1[:, csl], s1[:, csl])
                            keep_warm()
                            for sc in range(csl.start // P, csl.stop // P):
                                ssl = slice(sc * P, (sc + 1) * P)
                                for par, rp in ((0, rg), (1, r1)):
                                    ps_b = ps.tile([P, P], F32, tag="ps",
                                                   bufs=4, name="ps_b")
                                    nc.tensor.matmul(
                                        ps_b[0:HD, :],
                                        ones32,
                                        rp[0:1, ssl],
                                        start=True, stop=True)
                                    asl = attn[hp][par * HD:(par + 1) * HD,
                                                   ssl]
                                    nc.vector.tensor_mul(
                                        asl,
                                        av_st[par * HD:(par + 1) * HD, ssl],
                                        ps_b[0:HD, :])
                    else:
                        nc.vector.tensor_copy(av_st[0:HD, :],
                                              ps_av[0][0:HD, :])
                        nc.scalar.copy(av_st[HD:P, :], ps_av[1][0:HD, :])
                        nc.vector.tensor_copy(sg[0:1, :],
                                              ps_av[0][HD:HD + 1, :])
                        nc.vector.tensor_copy(s1, ps_av[1][HD:HD + 1, :])
                        nc.sync.dma_start(sg[1:2, :], s1)  # gather row 1
                        nc.vector.reciprocal_approx_fast(rg, sg)
                        rgb = work.tile([2, QCW], BF16, tag="recgb", bufs=2,
                                        name="rec_gb")
                        nc.vector.tensor_copy(rgb, rg)
                        for par in range(2):
                            asl = attn[hp][par * HD:(par + 1) * HD, :]
                            bounce = dr.tile([1, QCW], BF16, tag="bounce",
                                             bufs=4, name="bounce")
                            nc.sync.dma_start(bounce, rgb[par:par + 1, :])
                            nc.sync.dma_start(
                                asl, bounce.to_broadcast([HD, QCW]))
                            nc.vector.tensor_mul(
                                asl, av_st[par * HD:(par + 1) * HD, :], asl)
                while fq:
                    fq.pop(0)()
                return attn

            def _emit_av(hp, ps_av, pend, nkb):
                kb, wexp2, jofs, w = pend
                for par in range(2):
                    h = 2 * hp + par
                    nc.tensor.matmul(
                        ps_av[par][0:HD + 1, jofs:QCW],
                        v_sb[kb][:, h * (HD + 1):(h + 1) * (HD + 1)],
                        wexp2[:, par * QCW:par * QCW + w],
                        start=(kb == 0), stop=(kb == nkb - 1))

            # emission schedule: q0/k0 projection inline, the rest of
            # chunk 0's qkv rides inside attention(0) (it only needs x
            # chunk 0, which has landed); qkv(1) runs inline between
            # attention(0) and (1) because its x arrives too late to be
            # filler (an in-order PE queue stalls on a filler whose DMA is
            # pending).  Later chunks' x always lands in time, so qkv(2),
            # qkv(3) and the out-projections interleave as filler.
            qk_unit(0, 0)()
            qk_unit(0, 2)()
            attns = []
            for tcg in range(NQC):
                if tcg == 0:
                    filler = ([qk_unit(0, 1), qk_unit(0, 3)]
                              + [v_unit(0, tbl) for tbl in range(4)])
                elif tcg == 1:
                    filler = qkv_units(2) + outproj_units(0, attns[0])
                elif tcg == 2:
                    filler = qkv_units(3) + outproj_units(1, attns[1])
                else:
                    filler = outproj_units(2, attns[2])
                attn = emit_attention(tcg, filler, last=(tcg == NQC - 1))
                attns.append(attn)
                if tcg == 0:
                    for u in qkv_units(1):
                        u()
            for u in outproj_units(NQC - 1, attns[3], final=True):
                keep_warm()
                u()

    nc.compile()
    return nc


_PROGRAM = None


def _get_program():
    global _PROGRAM
    if _PROGRAM is None:
        _PROGRAM = build_program()
    return _PROGRAM


def make_in_maps(x, w_qkv, w_out):
    mask = np.triu(np.ones((P, P), dtype=np.float32))  # keep k<=q: i<=j
    in_maps = []
    for core in range(NCORES):
        b, p = core // HPC, core % HPC
        h0 = p * HPC * HD                       # first head col offset (256*p)
        in_maps.append({
            "xT": np.ascontiguousarray(x[b].T).astype(BF16_NP),
            "wqk": np.ascontiguousarray(np.concatenate(
                [w_qkv[:, h0:h0 + HPC * HD],
                 w_qkv[:, C + h0:C + h0 + HPC * HD]], axis=1)).astype(BF16_NP),
            "wv": np.ascontiguousarray(
                w_qkv[:, 2 * C + h0:2 * C + h0 + HPC * HD]).astype(BF16_NP),
            "wo": np.ascontiguousarray(w_out[h0:h0 + HPC * HD, :]).astype(BF16_NP),
            "mask": mask.astype(BF16_NP),
            "ones": np.ones((P, HD), dtype=BF16_NP),
        })
    return in_maps


def kernel(x, w_qkv, w_out):
    x = np.asarray(x, dtype=np.float32)
    w_qkv = np.asarray(w_qkv, dtype=np.float32)
    w_out = np.asarray(w_out, dtype=np.float32)
    nc = _get_program()
    res = bass_utils.run_bass_kernel_spmd(nc, make_in_maps(x, w_qkv, w_out),
                                          core_ids=list(range(NCORES)))
    y = np.zeros((B, T, C), dtype=np.float32)
    for core in range(NCORES):
        y[core // HPC] += res.results[core]["y"].astype(np.float32)
    return y


# revision 26
# speedup vs baseline: 1.0146x; 1.0146x over previous
"""Causal self-attention Trainium2 kernel (8 NeuronCores, SPMD).

Sharding: data-parallel over batch (B=2) x tensor-parallel over heads
(16 heads -> 4 per core).  core c: batch c//4, heads 4*(c%4) .. +4.
Each core computes qkv projection for its heads, causal attention, and a
partial out-projection; the host sums the 4 head-group partials per batch.

Layout notes:
  - Host passes x[b] pre-transposed (C, T) so the contraction dim C is
    partition-major for the qkv matmuls (PE contracts over partitions).
  - q,k are produced transposed (head_dim, T); v in natural (T, head_dim)
    with a ones column appended so the AV matmul also emits softmax row
    sums (row 64 of the PSUM accumulator).
  - scores are computed transposed (k, q) so the exp'd weights feed the
    AV matmul directly as the moving operand.
  - All matmul operands are bfloat16 (full-rate PE with fast weight
    load; fp32r pays a slow LDWEIGHTS and a 4x penalty at moving<256).
    PSUM accumulation stays fp32; rel err ~5e-3 vs the 2e-2 gate.
  - Static inputs live in consolidated tiles loaded with a few merged
    3D-AP DMAs (a DMA-issue instruction costs ~0.6us of engine time, so
    40 small loads would serialize the startup).
  - The PE instruction queue is in-order, so PE filler units are only
    scheduled into attention chunks whose input DMAs have landed.
  - Causal masking multiplies the exp'd diagonal blocks by a 0/1
    triangle on the (otherwise idle) GpSimd engine.
  - The partial y is stored bf16 (half the store traffic); the host
    upcasts and reduces in fp32.
"""

import numpy as np
import ml_dtypes

import concourse.bass as bass
import concourse.mybir as mybir
import concourse.tile as tile
from concourse import bacc
from concourse import bass_utils

# Problem shape (hardcoded per spec)
B, T, C = 2, 2048, 1024
NH, HD = 16, 64
NCORES = 8
HPC = 4                      # heads per core
P = 128                      # partitions
CB = C // P                  # 8 contraction blocks
QCW = 512                    # query chunk width
NQC = T // QCW               # 4 query chunks
NKB = T // P                 # 16 key blocks
SCALE = 1.0 / 8.0            # 1/sqrt(HD)
WQKC = 2 * HPC * HD          # wqk columns (512)
WVC = HPC * HD               # wv columns (256)
VW = HPC * (HD + 1)          # v tile width incl. ones cols (260)

F32 = mybir.dt.float32
BF16 = mybir.dt.bfloat16
EXP = mybir.ActivationFunctionType.Exp
BF16_NP = ml_dtypes.bfloat16


def build_program():
    nc = bacc.Bacc("TRN2", target_bir_lowering=False, debug=False,
                   num_devices=NCORES)

    xT = nc.dram_tensor("xT", [C, T], BF16, kind="ExternalInput").ap()
    wqk = nc.dram_tensor("wqk", [C, WQKC], BF16, kind="ExternalInput").ap()
    wv = nc.dram_tensor("wv", [C, WVC], BF16, kind="ExternalInput").ap()
    wo = nc.dram_tensor("wo", [HPC * HD, C], BF16, kind="ExternalInput").ap()
    mask = nc.dram_tensor("mask", [P, P], BF16, kind="ExternalInput").ap()
    ones = nc.dram_tensor("ones", [P, HD], BF16, kind="ExternalInput").ap()
    y = nc.dram_tensor("y", [T, C], BF16, kind="ExternalOutput").ap()

    # slab-major dram views for merged loads: [p, cb, cols]
    xT_v = xT.rearrange("(cb p) t -> p cb t", p=P)
    wqk_v = wqk.rearrange("(cb p) c -> p cb c", p=P)
    wv_v = wv.rearrange("(cb p) c -> p cb c", p=P)
    wo_v = wo.rearrange("(hp p) c -> p hp c", p=P)

    with tile.TileContext(nc) as tc:
        with tc.tile_pool(name="sb", bufs=1) as sb, \
             tc.tile_pool(name="work", bufs=1) as work, \
             tc.tile_pool(name="dr", bufs=1, space="DRAM") as dr, \
             tc.tile_pool(name="ps", bufs=1, space="PSUM") as ps:

            # ---- consolidated static tiles
            xT_all = sb.tile([P, CB * T], BF16, tag="xT", bufs=1,
                             name="xT_all")
            xT_av = xT_all.rearrange("p (cb t) -> p cb t", t=T)
            xT_sb = [xT_all[:, cb * T:(cb + 1) * T] for cb in range(CB)]
            wqk_all = sb.tile([P, CB * WQKC], BF16, tag="wqk", bufs=1,
                              name="wqk_all")
            wqk_av = wqk_all.rearrange("p (cb c) -> p cb c", c=WQKC)
            wqk_sb = [wqk_all[:, cb * WQKC:(cb + 1) * WQKC] for cb in range(CB)]
            wv_all = sb.tile([P, CB * WVC], BF16, tag="wv", bufs=1,
                             name="wv_all")
            wv_av = wv_all.rearrange("p (cb c) -> p cb c", c=WVC)
            wv_sb = [wv_all[:, cb * WVC:(cb + 1) * WVC] for cb in range(CB)]
            wo_all = sb.tile([P, 2 * C], BF16, tag="wo", bufs=1,
                             name="wo_all")
            wo_av = wo_all.rearrange("p (hp c) -> p hp c", c=C)
            wo_sb = [wo_all[:, hp * C:(hp + 1) * C] for hp in range(2)]

            # ---- startup loads: the first attention chunk needs wqk and
            # xT cols 0:512; stream those in 0.25MB quarters round-robined
            # over all three DMA queues so arrival is progressive and no
            # single queue serializes the transfers.
            rr = [nc.sync, nc.scalar, nc.gpsimd]
            for q in range(4):
                cbs = slice(2 * q, 2 * q + 2)
                rr[(2 * q) % 3].dma_start(wqk_av[:, cbs, :],
                                          wqk_v[:, cbs, :])
                rr[(2 * q + 1) % 3].dma_start(xT_av[:, cbs, 0:QCW],
                                              xT_v[:, cbs, 0:QCW])
            # then, in priority order: v weights, x chunk 1, out weights,
            # constants, x chunk 2; x chunk 3 is prefetched after attention
            # chunk 0 is emitted
            nc.gpsimd.dma_start(wv_av, wv_v)
            nc.scalar.dma_start(xT_av[:, :, QCW:2 * QCW],
                                xT_v[:, :, QCW:2 * QCW])
            nc.sync.dma_start(wo_av, wo_v)
            ones_sb = sb.tile([P, HD], BF16, tag="ones", bufs=1)
            nc.sync.dma_start(ones_sb, ones)
            mask_sb = sb.tile([P, P], BF16, tag="mask", bufs=1)
            nc.sync.dma_start(mask_sb, mask)
            # x chunks 2+3 as one load: 2KB contiguous runs transfer more
            # efficiently, and both land long before their consumers
            nc.scalar.dma_start(xT_av[:, :, 2 * QCW:4 * QCW],
                                xT_v[:, :, 2 * QCW:4 * QCW])
            # fp32 ones row for the endgame K=1 broadcast matmuls
            ones32 = sb.tile([1, HD], F32, tag="ones32", bufs=1)
            nc.vector.memset(ones32, 1.0)
            # warm the exp table early (one tiny activation)
            exp_warm = sb.tile([1, HD], F32, tag="expwarm", bufs=1)
            nc.scalar.activation(exp_warm, ones32, EXP)



            # ---- qkv projection ----
            # qk transposed, consolidated: col block jb*T+t;
            # jb 0,1 = q head pairs, 2,3 = k
            qk_all = sb.tile([P, 4 * T], BF16, tag="qk", bufs=1,
                             name="qk_all")
            qk_sb = [qk_all[:, jb * T:(jb + 1) * T] for jb in range(4)]
            # v natural per t-block, 4 heads x (64 v cols + ones col);
            # the ones cols never change -- set them once with a strided
            # memset instead of per-chunk copies
            v_all = sb.tile([P, NKB * VW], BF16, tag="v", bufs=1,
                            name="v_all")
            v_av = v_all.rearrange("p (tb h e) -> p tb h e", h=HPC, e=HD + 1)
            v_sb = [v_all[:, tb * VW:(tb + 1) * VW] for tb in range(NKB)]
            nc.gpsimd.memset(v_av[:, :, :, HD:HD + 1], 1.0)

            def qk_unit(tcg, jb):
                tsl = slice(tcg * QCW, (tcg + 1) * QCW)

                def emit():
                    ps_qk = ps.tile([P, QCW], F32, tag="ps", bufs=4,
                                    name="ps_qk")
                    for cb in range(CB):
                        nc.tensor.matmul(
                            ps_qk,
                            wqk_sb[cb][:, jb * P:(jb + 1) * P],
                            xT_sb[cb][:, tsl],
                            start=(cb == 0), stop=(cb == CB - 1))
                    nc.vector.tensor_copy(qk_sb[jb][:, tsl], ps_qk)
                return emit

            def v_unit(tcg, tbl):
                def emit():
                    tb = tcg * 4 + tbl
                    ps_v = ps.tile([P, HPC * HD], F32, tag="ps", bufs=4,
                                   name="ps_v")
                    for cb in range(CB):
                        nc.tensor.matmul(
                            ps_v,
                            xT_sb[cb][:, tb * P:(tb + 1) * P],
                            wv_sb[cb],
                            start=(cb == 0), stop=(cb == CB - 1))
                    nc.vector.tensor_copy(
                        v_av[:, tb, :, 0:HD],
                        ps_v.rearrange("p (h e) -> p h e", e=HD))
                return emit

            def qkv_units(tcg):
                return [qk_unit(tcg, jb) for jb in range(4)] + \
                       [v_unit(tcg, tbl) for tbl in range(4)]

            def outproj_units(qc, attn, copies="dve"):
                def op_unit(tbl):
                    def emit():
                        tb = qc * 4 + tbl
                        out_sb = work.tile([P, C], BF16, tag="outsb", bufs=2,
                                           name="out_sb")
                        if copies == "final":
                            # tail: the score PSUM pool is free, so borrow
                            # it (no WAR pressure on the rotating ps pool;
                            # a matmul output is limited to one bank, so
                            # each 512-column half is its own accumulation),
                            # split the copies across the idle ACT and DVE,
                            # store each half as soon as it lands
                            ps_o = ps.tile([P, 2 * QCW], F32, tag="ps2",
                                           bufs=2, name="ps_of")
                            for cob in range(2):
                                csl = slice(cob * QCW, (cob + 1) * QCW)
                                for hp in range(2):
                                    nc.tensor.matmul(
                                        ps_o[:, csl],
                                        attn[hp][:, tbl * P:(tbl + 1) * P],
                                        wo_sb[hp][:, csl],
                                        start=(hp == 0), stop=(hp == 1))
                                osl = out_sb[:, csl]
                                if cob == 0:
                                    nc.scalar.copy(osl, ps_o[:, csl])
                                else:
                                    nc.vector.tensor_copy(osl, ps_o[:, csl])
                                seng = nc.sync if (tbl + cob) % 2 == 0 \
                                    else nc.scalar
                                seng.dma_start(
                                    y[tb * P:(tb + 1) * P, csl], osl)
                            return
                        for cob in range(2):
                            ps_o = ps.tile([P, QCW], F32, tag="ps", bufs=4,
                                           name="ps_o")
                            for hp in range(2):
                                nc.tensor.matmul(
                                    ps_o,
                                    attn[hp][:, tbl * P:(tbl + 1) * P],
                                    wo_sb[hp][:, cob * QCW:(cob + 1) * QCW],
                                    start=(hp == 0), stop=(hp == 1))
                            osl = out_sb[:, cob * QCW:(cob + 1) * QCW]
                            if copies == "act":
                                # endgame filler: DVE is busy with the
                                # softmax normalization chain; route the
                                # PSUM drains through the idle ACT
                                nc.scalar.copy(osl, ps_o)
                            else:
                                nc.vector.tensor_copy(osl, ps_o)
                        nc.sync.dma_start(y[tb * P:(tb + 1) * P, :], out_sb)
                    return emit
                return [op_unit(tbl) for tbl in range(4)]

            def emit_attention(qc, filler, last=False, tail_units=()):
                """scores -> fused exp -> AV, with PE filler interleaved to
                keep TensorE dense (HAM warm) while ACT chews the exps."""
                attn = {}
                nkb = (qc + 1) * 4
                n_units = 2 * nkb
                fq = list(filler)
                credit = 0.0
                rate = len(fq) / n_units if n_units else 0.0
                for hp in range(2):
                    at = work.tile([P, QCW], BF16, tag="attn", bufs=6,
                                   name=f"attn_hp{hp}")
                    attn[hp] = at
                    ps_av = {}
                    for par in range(2):
                        ps_av[par] = ps.tile([P, QCW], F32, tag="ps", bufs=4,
                                             name="ps_av")
                    pend = []            # (kb, wexp2, jofs, w) awaiting AV
                    for kb in range(nkb):
                        kofs = kb - qc * 4
                        jofs = max(kofs, 0) * P
                        w = QCW - jofs
                        ps_s2 = ps.tile([P, 2 * QCW], F32, tag="ps2", bufs=2,
                                        name="ps_s2")
                        wexp2 = work.tile([P, 2 * QCW], BF16, tag="wexp2",
                                          bufs=5, name="wexp2")
                        diag = kofs >= 0
                        for par in range(2):
                            po = par * HD
                            nc.tensor.matmul(
                                ps_s2[:, par * QCW:par * QCW + w],
                                qk_sb[2 + hp][po:po + HD, kb * P:(kb + 1) * P],
                                qk_sb[hp][po:po + HD,
                                          qc * QCW + jofs:(qc + 1) * QCW],
                                start=True, stop=True)
                        sview = ps_s2.rearrange("p (g q) -> p g q", g=2)
                        wview = wexp2.rearrange("p (g q) -> p g q", g=2)
                        nc.scalar.activation(wview[:, :, 0:w],
                                             sview[:, :, 0:w], EXP, scale=SCALE)
                        if diag:
                            # zero the masked triangle on the idle GpSimd
                            # engine (frees the PE of 64 mask matmuls)
                            for par in range(2):
                                msl = wexp2[:, par * QCW:par * QCW + P]
                                nc.gpsimd.tensor_mul(msl, msl, mask_sb)
                        pend.append((kb, wexp2, jofs, w))
                        if len(pend) > 3:   # AV lags scores by 3 kb
                            _emit_av(hp, ps_av, pend.pop(0), nkb)
                        # drain PE filler to keep TensorE busy during exp
                        credit += rate
                        while credit >= 1.0 and fq:
                            fq.pop(0)()
                            credit -= 1.0
                    while pend:
                        _emit_av(hp, ps_av, pend.pop(0), nkb)
                        if fq:
                            fq.pop(0)()
                    # stage AV out of PSUM promptly so the banks recycle;
                    # heads land at their attn-aligned partition offsets
                    av_st = work.tile([P, QCW], BF16, tag="avst",
                                      bufs=4, name="av_st")
                    sg = work.tile([2, QCW], F32, tag="sumg", bufs=2,
                                   name="sums_g")
                    s1 = work.tile([1, QCW], F32, tag="sum1", bufs=2,
                                   name="sums_1")
                    rg = work.tile([2, QCW], F32, tag="recg", bufs=2,
                                   name="rec_g")
                    if last and hp == 1:
                        # endgame: ACT (done with exps) stages both av
                        # halves while DVE does sums + reciprocal; no DMA
                        # sums-gather, no bf16 casts; broadcast via K=1
                        # fp32 ones-row matmuls on the (idle) PE.  First
                        # 128 columns go alone so the out-projection tail
                        # starts immediately.
                        nc.scalar.copy(av_st[0:HD, :], ps_av[0][0:HD, :])
                        nc.scalar.copy(av_st[HD:P, :], ps_av[1][0:HD, :])
                        nc.vector.tensor_copy(sg[0:1, :],
                                              ps_av[0][HD:HD + 1, :])
                        nc.vector.tensor_copy(s1, ps_av[1][HD:HD + 1, :])
                        r1 = work.tile([1, QCW], F32, tag="rec1", bufs=1,
                                       name="rec_1")
                        tail = list(tail_units)
                        if tail:
                            # real PE work (outproj units of the previous
                            # chunk) fills the normalization latency and
                            # keeps the HAM clock gate warm for the tail
                            tail.pop(0)()
                        for csl in (slice(0, P), slice(P, QCW)):
                            nc.vector.reciprocal_approx_fast(
                                rg[0:1, csl], sg[0:1, csl])
                            nc.vector.reciprocal_approx_fast(
                                r1[:, csl], s1[:, csl])
                            if tail:
                                tail.pop(0)()
                            for sc in range(csl.start // P, csl.stop // P):
                                ssl = slice(sc * P, (sc + 1) * P)
                                for par, rp in ((0, rg), (1, r1)):
                                    ps_b = ps.tile([P, P], F32, tag="ps",
                                                   bufs=4, name="ps_b")
                                    nc.tensor.matmul(
                                        ps_b[0:HD, :],
                                        ones32,
                                        rp[0:1, ssl],
                                        start=True, stop=True)
                                    asl = attn[hp][par * HD:(par + 1) * HD,
                                                   ssl]
                                    nc.vector.tensor_mul(
                                        asl,
                                        av_st[par * HD:(par + 1) * HD, ssl],
                                        ps_b[0:HD, :])
                    else:
                        nc.vector.tensor_copy(av_st[0:HD, :],
                                              ps_av[0][0:HD, :])
                        nc.scalar.copy(av_st[HD:P, :], ps_av[1][0:HD, :])
                        nc.vector.tensor_copy(sg[0:1, :],
                                              ps_av[0][HD:HD + 1, :])
                        nc.vector.tensor_copy(s1, ps_av[1][HD:HD + 1, :])
                        nc.sync.dma_start(sg[1:2, :], s1)  # gather row 1
                        nc.vector.reciprocal_approx_fast(rg, sg)
                        rgb = work.tile([2, QCW], BF16, tag="recgb", bufs=2,
                                        name="rec_gb")
                        nc.vector.tensor_copy(rgb, rg)
                        for par in range(2):
                            asl = attn[hp][par * HD:(par + 1) * HD, :]
                            bounce = dr.tile([1, QCW], BF16, tag="bounce",
                                             bufs=4, name="bounce")
                            nc.sync.dma_start(bounce, rgb[par:par + 1, :])
                            nc.sync.dma_start(
                                asl, bounce.to_broadcast([HD, QCW]))
                            nc.vector.tensor_mul(
                                asl, av_st[par * HD:(par + 1) * HD, :], asl)
                while fq:
                    fq.pop(0)()
                return attn

            def _emit_av(hp, ps_av, pend, nkb):
                kb, wexp2, jofs, w = pend
                for par in range(2):
                    h = 2 * hp + par
                    nc.tensor.matmul(
                        ps_av[par][0:HD + 1, jofs:QCW],
                        v_sb[kb][:, h * (HD + 1):(h + 1) * (HD + 1)],
                        wexp2[:, par * QCW:par * QCW + w],
                        start=(kb == 0), stop=(kb == nkb - 1))

            # emission schedule: q0/k0 projection inline, the rest of
            # chunk 0's qkv rides inside attention(0) (it only needs x
            # chunk 0, which has landed); qkv(1) runs inline between
            # attention(0) and (1) because its x arrives too late to be
            # filler (an in-order PE queue stalls on a filler whose DMA is
            # pending).  Later chunks' x always lands in time, so qkv(2),
            # qkv(3) and the out-projections interleave as filler.
            qk_unit(0, 0)()
            qk_unit(0, 2)()
            attns = []
            for tcg in range(NQC):
                tail_units = ()
                if tcg == 0:
                    filler = ([qk_unit(0, 1), qk_unit(0, 3)]
                              + [v_unit(0, tbl) for tbl in range(4)])
                elif tcg == 1:
                    filler = qkv_units(2) + outproj_units(0, attns[0])
                elif tcg == 2:
                    filler = qkv_units(3) + outproj_units(1, attns[1])
                else:
                    filler = outproj_units(2, attns[2])[0:2]
                    tail_units = outproj_units(2, attns[2],
                                               copies="act")[2:4]
                attn = emit_attention(tcg, filler, last=(tcg == NQC - 1),
                                      tail_units=tail_units)
                attns.append(attn)
                if tcg == 0:
                    for u in qkv_units(1):
                        u()
            for u in outproj_units(NQC - 1, attns[3], copies="final"):
                u()

    nc.compile()
    return nc


_PROGRAM = None


def _get_program():
    global _PROGRAM
    if _PROGRAM is None:
        _PROGRAM = build_program()
    return _PROGRAM


def make_in_maps(x, w_qkv, w_out):
    mask = np.triu(np.ones((P, P), dtype=np.float32))  # keep k<=q: i<=j
    in_maps = []
    for core in range(NCORES):
        b, p = core // HPC, core % HPC
        h0 = p * HPC * HD                       # first head col offset (256*p)
        in_maps.append({
            "xT": np.ascontiguousarray(x[b].T).astype(BF16_NP),
            "wqk": np.ascontiguousarray(np.concatenate(
                [w_qkv[:, h0:h0 + HPC * HD],
                 w_qkv[:, C + h0:C + h0 + HPC * HD]], axis=1)).astype(BF16_NP),
            "wv": np.ascontiguousarray(
                w_qkv[:, 2 * C + h0:2 * C + h0 + HPC * HD]).astype(BF16_NP),
            "wo": np.ascontiguousarray(w_out[h0:h0 + HPC * HD, :]).astype(BF16_NP),
            "mask": mask.astype(BF16_NP),
            "ones": np.ones((P, HD), dtype=BF16_NP),
        })
    return in_maps


def kernel(x, w_qkv, w_out):
    x = np.asarray(x, dtype=np.float32)
    w_qkv = np.asarray(w_qkv, dtype=np.float32)
    w_out = np.asarray(w_out, dtype=np.float32)
    nc = _get_program()
    res = bass_utils.run_bass_kernel_spmd(nc, make_in_maps(x, w_qkv, w_out),
                                          core_ids=list(range(NCORES)))
    y = np.zeros((B, T, C), dtype=np.float32)
    for core in range(NCORES):
        y[core // HPC] += res.results[core]["y"].astype(np.float32)
    return y
